# revision 3
# baseline (speedup 1.0000x reference)
"""Weighted per-class dice loss on 8 trn2 NeuronCores (batch-sharded).

v10 = v8 + 3 PSUM sets (elastic extract chain) + half-split class 17:
  - no on-chip fold: the [128, 57] accumulator tile is DMA'd out and
    folded on host (saves the fold->copy->DMA hop chain);
  - class 18's PE groups interleave at half-tile granularity so only
    ~half a group runs after the last DMA byte;
  - Block(no_gpsimd_drain=True) skips the expensive SWDGE drain at
    NEFF exit (all SWDGE DMAs are provably complete by then).

Scheme (per class c):
  tsum_c : DVE stt  mw_c = (L==c)*W, accum -> tsum   (1x, 4.4 us)
  psum_c : PE: 32 chunk-matmuls pred_c[:,k]^T @ W[:,k] into a [128,128]
           PSUM tile; diag holds per-column dots.
  inter_c: same against mw_c.
  DVE extracts diags via 285 ns stt vs a host identity, one PE group
  after the stop-matmul (PSUM writeback drain).
pred arrives via SWDGE cast-DMAs (f32 HBM -> bf16 SBUF, ~390 GB/s);
W/L/ident ride the HWDGE sync ring with ACT casts.
"""

import numpy as np
import ml_dtypes

import concourse.bass as bass
from concourse import mybir
from concourse.bass_utils import run_bass_kernel_spmd

C = 19
P = 128
FCOL_FULL = 4096
SMOOTH = 1.0
PRED_RING = 6
MW_RING = 4

F = mybir.dt.float32
BF = mybir.dt.bfloat16


def build_nc(fcol: int = FCOL_FULL) -> bass.Bass:
    nchunk = fcol // P
    half = fcol // 2
    hchunk = nchunk // 2
    nc = bass.Bass()
    pred = nc.dram_tensor("pred", [C, P, fcol], F, kind="ExternalInput")
    tgt = nc.dram_tensor("target", [2, P, fcol], F, kind="ExternalInput")
    ident_in = nc.dram_tensor("ident", [P, P], BF, kind="ExternalInput")
    partials = nc.dram_tensor("partials", [P, 3 * C], F, kind="ExternalOutput")

    mult = mybir.AluOpType.mult
    is_eq = mybir.AluOpType.is_equal

    from contextlib import ExitStack

    _es = ExitStack()
    with _es:
        def sb(name, shape, dt):
            return _es.enter_context(nc.sbuf_tensor(name, shape, dt))

        lf = sb("lf", [P, fcol], F); wf = sb("wf", [P, fcol], F)
        lb = sb("lb", [P, fcol], BF); wb = sb("wb", [P, fcol], BF)
        ring = [sb(f"pb{i}", [P, fcol], BF) for i in range(PRED_RING)]
        mws = [sb(f"mw{i}", [P, fcol], BF) for i in range(MW_RING)]
        ident = sb("identsb", [P, P], BF)
        junk = sb("junk", [P, P], BF)
        accs = sb("accs", [P, 3 * C], F); ones = sb("ones", [P, 1], F)
        psA = [_es.enter_context(nc.psum_tensor(f"psA{i}", [P, P], F))
               for i in range(3)]
        psB = [_es.enter_context(nc.psum_tensor(f"psB{i}", [P, P], F))
               for i in range(3)]
        psD = _es.enter_context(nc.psum_tensor("psD", [1, 1], F))

        def sem(name):
            return _es.enter_context(nc.semaphore(name))

        dsem = sem("dsem")    # gpsimd pred DMAs in order (+16 each)
        ssem = sem("ssem")    # sync ring: W, L (+16 each)
        isem = sem("isem")    # ident DMA (sync)
        act_sem = sem("act_sem")  # wb, lb casts
        vsem = sem("vsem")    # every DVE op (+1 each)
        pe_sem = sem("pe_sem")  # A_c -> 2c+1, B_c -> 2c+2, barrier -> 39
        osem = sem("osem")
        block = _es.enter_context(nc.Block(no_gpsimd_drain=True))

        def dsem_pred(c: int) -> int:
            return 16 * (c + 1)  # full tile of class c landed (c <= 17)

        DSEM_18A = 16 * 19
        DSEM_18B = 16 * 20

        # --- DVE program schedule (indices for cross-engine waits) -----
        mw_done = {}
        extrB_done = {}
        v = 1  # memset
        for i in range(min(MW_RING, C)):
            v += 1
            mw_done[i] = v
        for c in range(C):
            v += 2
            extrB_done[c] = v
            if c + MW_RING < C:
                v += 1
                mw_done[c + MW_RING] = v
        V_TOTAL = v

        @block.gpsimd
        def _(g: bass.BassEngine):
            for c in range(C - 2):
                if c >= PRED_RING:
                    g.wait_ge(pe_sem, 2 * (c - PRED_RING) + 2)
                g.dma_start(out=ring[c % PRED_RING][:], in_=pred[c]).then_inc(
                    dsem, 16)
            c = C - 2
            g.wait_ge(pe_sem, 2 * (c - PRED_RING) + 2)
            slot = ring[c % PRED_RING]
            for i in range(2):
                g.dma_start(out=slot[:, i * half : (i + 1) * half],
                            in_=pred[c, :, i * half : (i + 1) * half]).then_inc(
                    dsem, 16)
            c = C - 1
            g.wait_ge(pe_sem, 2 * (c - PRED_RING) + 2)
            slot = ring[c % PRED_RING]
            q = fcol // 4
            for i in range(4):
                g.dma_start(out=slot[:, i * q : (i + 1) * q],
                            in_=pred[c, :, i * q : (i + 1) * q]).then_inc(
                    dsem, 16)

        @block.sync
        def _(s: bass.BassEngine):
            s.dma_start(out=wf[:], in_=tgt[1]).then_inc(ssem, 16)
            s.dma_start(out=lf[:], in_=tgt[0]).then_inc(ssem, 16)
            s.dma_start(out=ident[:], in_=ident_in[:]).then_inc(isem, 16)

        @block.scalar
        def _(sc: bass.BassEngine):
            sc.wait_ge(ssem, 16)
            sc.copy(out=wb[:], in_=wf[:]).then_inc(act_sem, 1)
            sc.wait_ge(ssem, 32)
            sc.copy(out=lb[:], in_=lf[:]).then_inc(act_sem, 1)
            # accumulator DMA'd out raw; host does the partition fold
            sc.wait_ge(vsem, V_TOTAL)
            sc.dma_start(out=partials[:], in_=accs[:]).then_inc(osem, 16)

        @block.vector
        def _(vec: bass.BassEngine):
            vec.memset(ones[:], 1.0).then_inc(vsem, 1)

            def mw_op(c: int):
                if c >= MW_RING:
                    vec.wait_ge(pe_sem, 2 * (c - MW_RING) + 2)
                vec.scalar_tensor_tensor(
                    out=mws[c % MW_RING][:], in0=lb[:], scalar=float(c),
                    in1=wb[:], op0=is_eq, op1=mult,
                    accum_out=accs[:, 2 * C + c : 2 * C + c + 1],
                ).then_inc(vsem, 1)

            vec.wait_ge(act_sem, 2)  # wb and lb ready
            for i in range(min(MW_RING, C)):
                mw_op(i)
            vec.wait_ge(isem, 16)
            for c in range(C):
                s = c % 3
                # read PSUM one PE group after the stop-matmul so the
                # accumulation writeback has fully drained
                vec.wait_ge(pe_sem, 2 * c + 3)
                vec.scalar_tensor_tensor(
                    out=junk[:], in0=psA[s][:], scalar=1.0, in1=ident[:],
                    op0=mult, op1=mult,
                    accum_out=accs[:, c : c + 1]).then_inc(vsem, 1)
                vec.scalar_tensor_tensor(
                    out=junk[:], in0=psB[s][:], scalar=1.0, in1=ident[:],
                    op0=mult, op1=mult,
                    accum_out=accs[:, C + c : C + c + 1]).then_inc(vsem, 1)
                if c + MW_RING < C:
                    mw_op(c + MW_RING)

        @block.tensor
        def _(t: bass.BassEngine):
            t.wait_ge(act_sem, 1)  # wb
            for c in range(C - 2):
                s = c % 3
                t.wait_ge(dsem, dsem_pred(c))
                if c >= 3:
                    t.wait_ge(vsem, extrB_done[c - 3])  # psA/psB set free
                pb = ring[c % PRED_RING]
                last = None
                for k in range(nchunk):
                    last = t.matmul(psA[s][:], pb[:, k * P : (k + 1) * P],
                                    wb[:, k * P : (k + 1) * P],
                                    start=(k == 0), stop=(k == nchunk - 1))
                last.then_inc(pe_sem, 1)
                t.wait_ge(vsem, mw_done[c])
                mw = mws[c % MW_RING]
                for k in range(nchunk):
                    last = t.matmul(psB[s][:], pb[:, k * P : (k + 1) * P],
                                    mw[:, k * P : (k + 1) * P],
                                    start=(k == 0), stop=(k == nchunk - 1))
                last.then_inc(pe_sem, 1)
            # class 17: half-tile granularity
            c = C - 2
            s = c % 3
            t.wait_ge(vsem, extrB_done[c - 3])
            t.wait_ge(vsem, mw_done[c])
            pb = ring[c % PRED_RING]
            mw = mws[c % MW_RING]
            lastA = lastB = None
            for hi in range(2):
                t.wait_ge(dsem, 16 * (18 + hi))
                for k in range(hi * hchunk, (hi + 1) * hchunk):
                    lastA = t.matmul(psA[s][:], pb[:, k * P : (k + 1) * P],
                                     wb[:, k * P : (k + 1) * P],
                                     start=(k == 0), stop=(k == nchunk - 1),
                                     skip_group_check=True)
                for k in range(hi * hchunk, (hi + 1) * hchunk):
                    lastB = t.matmul(psB[s][:], pb[:, k * P : (k + 1) * P],
                                     mw[:, k * P : (k + 1) * P],
                                     start=(k == 0), stop=(k == nchunk - 1),
                                     skip_group_check=True)
            lastA.then_inc(pe_sem, 1)
            lastB.then_inc(pe_sem, 1)
            # class 18: quarter-tile granularity so only a quarter group
            # trails the final DMA
            c = C - 1
            s = c % 3
            t.wait_ge(vsem, extrB_done[c - 3])
            t.wait_ge(vsem, mw_done[c])
            pb = ring[c % PRED_RING]
            mw = mws[c % MW_RING]
            qchunk = nchunk // 4
            lastA = lastB = None
            for qi in range(4):
                t.wait_ge(dsem, 16 * (20 + qi))
                for k in range(qi * qchunk, (qi + 1) * qchunk):
                    lastA = t.matmul(psA[s][:], pb[:, k * P : (k + 1) * P],
                                     wb[:, k * P : (k + 1) * P],
                                     start=(k == 0), stop=(k == nchunk - 1),
                                     skip_group_check=True)
                for k in range(qi * qchunk, (qi + 1) * qchunk):
                    lastB = t.matmul(psB[s][:], pb[:, k * P : (k + 1) * P],
                                     mw[:, k * P : (k + 1) * P],
                                     start=(k == 0), stop=(k == nchunk - 1),
                                     skip_group_check=True)
            lastA.then_inc(pe_sem, 1)
            lastB.then_inc(pe_sem, 1)
            # barrier matmul: completion implies the last B writeback has
            # drained (in-order PE), unblocking the final extracts
            t.matmul(psD[:], ones[:], ones[:], start=True, stop=True).then_inc(
                pe_sem, 1)

    return nc


def make_in_maps(pred: np.ndarray, target: np.ndarray) -> list:
    B, C_, H, Wd = pred.shape
    n = H * Wd
    fcol = n // P
    pred_r = np.ascontiguousarray(pred.reshape(B, C_, P, fcol).astype(np.float32))
    tgt_r = np.ascontiguousarray(target.reshape(B, 2, P, fcol).astype(np.float32))
    ident = np.eye(P, dtype=np.float32).astype(ml_dtypes.bfloat16)
    return [{"pred": pred_r[i], "target": tgt_r[i], "ident": ident}
            for i in range(B)]


def _combine(parts: np.ndarray) -> np.ndarray:
    # parts: [B, P, 3C] per-partition accumulators (accepts flattened too)
    parts = parts.reshape(parts.shape[0], P, 3 * C)
    tot = parts.astype(np.float64).sum(axis=(0, 1))
    psum, inter, tsum = tot[0:C], tot[C : 2 * C], tot[2 * C : 3 * C]
    dice = (2.0 * inter + SMOOTH) / (psum + tsum + SMOOTH)
    loss = np.sum(1.0 - dice) / C
    return np.asarray(loss, dtype=np.float32)


def kernel(pred: np.ndarray, target: np.ndarray) -> np.ndarray:
    B = pred.shape[0]
    fcol = pred.shape[2] * pred.shape[3] // P
    nc = build_nc(fcol)
    in_maps = make_in_maps(pred, target)
    res = run_bass_kernel_spmd(nc, in_maps, list(range(B))).results
    parts = np.stack([r["partials"] for r in res])
    return _combine(parts)
